# revision 5
# baseline (speedup 1.0000x reference)
"""Conditional contrastive loss on 8 TRN2 NeuronCores — transposed design.

Per core (512 own rows of inst_embed "x" / proxy "p", both matrices):
  - simT[j, i] with j on PSUM partitions: per j-block, lhsT = xn^T[jb]
    (stationary), rhs = own pn^T / xn^T columns (moving, 512 wide), fp8
    DoubleRow, K=512 = 2 DR instructions.
  - exp((simT - margin)/T - SHIFT) on ScalarE straight out of PSUM into
    fp8 SBUF z tiles, 4 j-blocks ([128, 2048]) per ACTIVATE.  fp8e4 here
    is IEEE-ish with max finite 240; quantized-renormalized unit vectors
    keep |sim| <= ~1.01 so z_max = e^(10.1-4.9) = 181 < 240.  The
    e^{-SHIFT} factor cancels in log(den) - log(num).
  - num/den on the tensor engine: mask[i,j] = nm[labels[i], j] is
    rank-100, so numC = [nm; ones] @ Z^T is an fp8 DR matmul stream
    contracting over j.  Row 100 (ones row) gives den; the host gathers
    num[i] = numC[labels[i], i].
  - PSUM is exactly 2 x [128, 2048] act groups (8 banks).  The num
    matmuls for each 4-jb group accumulate into a borrowed bank of the
    group's own (already act-read) PSUM tile; the DVE then adds the
    partial into an SBUF f32 accumulator.  No persistent num bank, no
    final PSUM->SBUF copy; the out DMA reads the SBUF accumulator.
  - Engine budget: PE 34.6us, ScalarE 30.7us, DVE 13us.
"""
import numpy as np
import ml_dtypes

import concourse.bacc as bacc
import concourse.tile as tile
from concourse import mybir, bass_utils

N_FULL = 4096
D = 512
N_CORES = 8
RP = N_FULL // N_CORES     # own rows per core = 512
P = 128
KC = D // P                # 4 contraction chunks of 128
JB = N_FULL // P           # 32 j-blocks
NPAIR = JB // 2            # 16 DR pairs for the num matmul
GRP = 4                    # j-blocks per ACTIVATE group
NGRP = JB // GRP           # 8 groups per matrix
CP = 112                   # classes 0..99, ones row at 100, zero pad
SHIFT = 4.9

F32 = mybir.dt.float32
BF16 = mybir.dt.bfloat16
F8 = mybir.dt.float8e4
AF = mybir.ActivationFunctionType
ALU = mybir.AluOpType
DR = mybir.MatmulPerfMode.DoubleRow

# xq DMA pieces (j-blocks per piece)
XQ_SPLIT = [3, 8, 8, 13]
# act-group sizes (j-blocks) per matrix: 3-jb groups keep the act
# (1536+312 cyc) inside the PE window (7.5 x 518 cyc); a small final group
# shortens the serial act->num->copy->DMA tail
GRPS0 = [3] * 10 + [2]
GRPS1 = [3] * 10 + [2]

_CACHE = {}


def _prefix(sizes):
    o = 0
    for n in sizes:
        yield o, n
        o += n


def _build(inv_t: float, bias_val: float):
    nc = bacc.Bacc("TRN2", target_bir_lowering=False, debug=False,
                   num_devices=N_CORES)

    # xq[p, jb, k, j'] = xn^T[k*128+p, jb*128+j']  (full xn)
    xq = nc.dram_tensor("xq", [P, JB * KC * P], F8, kind="ExternalInput")
    # wcc[p, m, k, i]: m=0 own pn^T cols, m=1 own xn^T cols
    wcc = nc.dram_tensor("wcc", [P, 2 * KC * RP], F8, kind="ExternalInput")
    # nmq[p, pair, kb, c] = nmx[c, (2*pair+kb)*128+p], nmx = [nm; ones; 0]
    nmq = nc.dram_tensor("nmq", [P, NPAIR * 2 * CP], F8, kind="ExternalInput")
    out = nc.dram_tensor("out", [CP, 2 * RP], F32, kind="ExternalOutput")

    with tile.TileContext(nc) as tc:
        with (
            tc.tile_pool(name="xpool", bufs=1) as xpool,
            tc.tile_pool(name="wpool", bufs=1) as wpool,
            tc.tile_pool(name="zpool", bufs=1) as zpool,
            tc.tile_pool(name="small", bufs=1) as small,
            tc.tile_pool(name="ps", bufs=2, space="PSUM") as pspool,
            tc.tile_pool(name="nps", bufs=2, space="PSUM") as npspool,
        ):
            # ---- constants ----
            dummy = small.tile([P, 1], F32, name="dummy")
            nc.vector.memset(dummy[:], 0.0)
            nc.scalar.activation(dummy[:], dummy[:], AF.Exp)
            bias_t = small.tile([P, 1], F32, name="bias_t")
            nc.vector.memset(bias_t[:], bias_val)
            zeros_w = small.tile([P, P], BF16, name="zeros_w")
            nc.vector.memset(zeros_w[:], 0.0)
            zeros_r = small.tile([P, 512], BF16, name="zeros_r")
            nc.vector.memset(zeros_r[:], 0.0)

            # ---- input tiles + DMAs, serial Sync kicks in order of first
            # use (per-DMA landing has a ~3.5us fixed floor, so fewer,
            # bigger pieces beat fine-grained splitting)
            xqt = [xpool.tile([P, n, KC, P], F8, name=f"xq{i}")
                   for i, n in enumerate(XQ_SPLIT)]
            pnt = wpool.tile([P, KC, RP], F8, name="pnt")
            xnt = wpool.tile([P, KC, RP], F8, name="xnt")
            nmqa = wpool.tile([P, 4, 2, CP], F8, name="nmqa")
            nmqb = wpool.tile([P, NPAIR - 4, 2, CP], F8, name="nmqb")
            zt = zpool.tile([P, 2 * JB, RP], F8, name="zt")

            offs = []
            o = 0
            for n in XQ_SPLIT:
                offs.append(o)
                o += n
            xq3 = xq.ap().rearrange("p (jb k j) -> p jb (k j)", jb=JB, k=KC)
            wc3 = wcc.ap().rearrange("p (m r) -> p m r", m=2)
            nm3 = nmq.ap().rearrange("p (u kb c) -> p u (kb c)",
                                     u=NPAIR, kb=2)

            def kick_xq(i):
                nc.sync.dma_start(
                    xqt[i][:].rearrange("p a b c -> p a (b c)"),
                    xq3[:, offs[i]:offs[i] + XQ_SPLIT[i]])

            nc.sync.dma_start(
                pnt[:].rearrange("p a b -> p (a b)"), wc3[:, 0])
            kick_xq(0)
            nc.sync.dma_start(
                nmqa[:].rearrange("p a b c -> p a (b c)"), nm3[:, 0:4])
            kick_xq(1)
            nc.sync.dma_start(
                nmqb[:].rearrange("p a b c -> p a (b c)"), nm3[:, 4:NPAIR])
            kick_xq(2)
            kick_xq(3)
            nc.sync.dma_start(
                xnt[:].rearrange("p a b -> p (a b)"), wc3[:, 1])

            def xq_slice(jb, b):
                for i, n in enumerate(XQ_SPLIT):
                    if jb < offs[i] + n:
                        return xqt[i][:, jb - offs[i], 2 * b:2 * b + 2, :]
                raise AssertionError

            def nm_slice(pair):
                if pair < 4:
                    return nmqa[:, pair, :, :]
                return nmqb[:, pair - 4, :, :]

            # units: (m, jb0, njb) act groups in stream order
            units = ([(0, j, n) for j, n in _prefix(GRPS0)]
                     + [(1, j, n) for j, n in _prefix(GRPS1)])
            num_ps = [npspool.tile([P, 512], F32, name=f"num{m}", tag="num")
                      for m in range(2)]
            ncopy = small.tile([P, 2, RP], F32, name="ncopy")

            def emit_num(idx):
                """num matmuls for all j-pairs newly completed by unit
                idx's act, into the matrix's persistent PSUM bank; copy
                out + DMA when the matrix completes."""
                m, jb0, njb = units[idx]
                pr0 = jb0 // 2            # pairs done before this unit
                prn = (jb0 + njb) // 2    # pairs done after it
                for pair in range(pr0, prn):
                    nc.tensor.matmul(
                        num_ps[m][:CP, :], nm_slice(pair),
                        zt[:, m * JB + 2 * pair:m * JB + 2 * pair + 2, :],
                        start=(pair == 0), stop=(pair == NPAIR - 1),
                        perf_mode=DR)
                if idx + 1 == len(units) or units[idx + 1][0] != m:
                    nc.vector.tensor_copy(ncopy[:CP, m, :], num_ps[m][:CP, :])
                    nc.sync.dma_start(out.ap()[:, m * RP:(m + 1) * RP],
                                      ncopy[:CP, m, :])

            prev = None
            for idx, (m, jb0, njb) in enumerate(units):
                ps = pspool.tile([P, 1536], F32, name=f"ps_{idx}", tag="ps")
                if idx == 0:
                    # HAM warm-up while the first DMAs land; must run
                    # contiguously into the real stream
                    for w in range(9):
                        nc.tensor.matmul(
                            ps[:, 0:512], zeros_w[:], zeros_r[:],
                            start=(w == 0), stop=(w == 8))
                for jl in range(njb):
                    jb = jb0 + jl
                    for b in range(2):
                        wt = pnt if m == 0 else xnt
                        rhs = wt[:, 2 * b:2 * b + 2, :]
                        nc.tensor.matmul(
                            ps[:, jl * 512:(jl + 1) * 512],
                            xq_slice(jb, b), rhs,
                            start=(b == 0), stop=(b == 1),
                            perf_mode=DR)
                u0 = m * JB + jb0
                nc.scalar.activation(
                    zt[:, u0:u0 + njb, :].rearrange("p a b -> p (a b)"),
                    ps[:, 0:njb * 512], AF.Exp, bias=bias_t[:], scale=inv_t)
                # num work for the PREVIOUS unit: its act has had a full
                # matmul window to finish, so the in-order PE queue does
                # not stall on the scalar engine
                if prev is not None:
                    emit_num(prev)
                prev = idx
            emit_num(prev)

    nc.compile()
    return nc


def _norm8(a):
    """fp8-quantize a row-normalized matrix, renormalizing once after
    quantization so self-similarity stays ~1 (keeps exp in fp8 range)."""
    eps = 1e-8
    an = a / np.maximum(np.linalg.norm(a, axis=-1, keepdims=True), eps)
    a8 = an.astype(ml_dtypes.float8_e4m3).astype(np.float32)
    a8 /= np.maximum(np.linalg.norm(a8, axis=-1, keepdims=True), eps)
    return a8.astype(ml_dtypes.float8_e4m3)


def make_in_maps(x, p, nmf, lab):
    xnT = np.ascontiguousarray(_norm8(x).T)   # [512, 4096] fp8
    pnT = np.ascontiguousarray(_norm8(p).T)
    # xq[p, jb, k, j'] = xnT[k*128+p, jb*128+j']
    xq = np.ascontiguousarray(
        xnT.reshape(KC, P, JB, P).transpose(1, 2, 0, 3)).reshape(P, -1)
    nmx = np.zeros((CP, N_FULL), np.float32)
    nmx[:100] = nmf
    nmx[100] = 1.0
    nmq = np.ascontiguousarray(
        nmx.T.reshape(NPAIR, 2, P, CP).transpose(2, 0, 1, 3)
    ).astype(ml_dtypes.float8_e4m3).reshape(P, -1)

    in_maps = []
    for c in range(N_CORES):
        rows = slice(c * RP, (c + 1) * RP)
        # wcc[p, m, k, i] = (pn|xn)T[k*128+p, rows]
        pncol = np.ascontiguousarray(
            pnT[:, rows].reshape(KC, P, RP).transpose(1, 0, 2)).reshape(P, -1)
        xncol = np.ascontiguousarray(
            xnT[:, rows].reshape(KC, P, RP).transpose(1, 0, 2)).reshape(P, -1)
        in_maps.append({
            "xq": xq,
            "wcc": np.concatenate([pncol, xncol], axis=1),
            "nmq": nmq,
        })
    return in_maps


def kernel(inst_embed, proxy, negative_mask, labels, temperature, margin):
    t = float(np.asarray(temperature))
    m = float(np.asarray(margin))
    inv_t = 1.0 / t
    bias_val = -m / t - SHIFT

    key = (t, m)
    if key not in _CACHE:
        _CACHE[key] = _build(inv_t, bias_val)
    nc = _CACHE[key]

    x = np.asarray(inst_embed, dtype=np.float32)
    p = np.asarray(proxy, dtype=np.float32)
    nmf = np.asarray(negative_mask, dtype=np.float32)
    lab = np.asarray(labels).astype(np.int64)

    in_maps = make_in_maps(x, p, nmf, lab)
    res = bass_utils.run_bass_kernel_spmd(nc, in_maps,
                                          core_ids=list(range(N_CORES)))
    idx = np.arange(RP)
    tot = 0.0
    for c in range(N_CORES):
        r = np.asarray(res.results[c]["out"]).astype(np.float64)
        lab_l = lab[c * RP:(c + 1) * RP]
        for m_i in range(2):
            blk = r[:, m_i * RP:(m_i + 1) * RP]
            den = blk[100, :]
            num = blk[lab_l, idx]
            tot += (np.log(den) - np.log(num)).sum()
    loss = -2.0 * np.log(t) + tot / N_FULL
    return np.float32(loss)


# revision 7
# speedup vs baseline: 1.2459x; 1.2459x over previous
"""Conditional contrastive loss on 8 TRN2 NeuronCores — transposed design.

Per core (512 own rows of inst_embed "x" / proxy "p", both matrices):
  - simT[j, i] with j on PSUM partitions: per j-block, lhsT = xn^T[jb]
    (stationary), rhs = own pn^T / xn^T columns (moving, 512 wide), fp8
    DoubleRow, K=512 = 2 DR instructions.
  - exp((simT - margin)/T - SHIFT) on ScalarE straight out of PSUM into
    fp8 SBUF z tiles, 4 j-blocks ([128, 2048]) per ACTIVATE.  fp8e4 here
    is IEEE-ish with max finite 240; quantized-renormalized unit vectors
    keep |sim| <= ~1.01 so z_max = e^(10.1-4.9) = 181 < 240.  The
    e^{-SHIFT} factor cancels in log(den) - log(num).
  - num/den on the tensor engine: mask[i,j] = nm[labels[i], j] is
    rank-100, so numC = [nm; ones] @ Z^T is an fp8 DR matmul stream
    contracting over j.  Row 100 (ones row) gives den; the host gathers
    num[i] = numC[labels[i], i].
  - PSUM is exactly 2 x [128, 2048] act groups (8 banks).  The num
    matmuls for each 4-jb group accumulate into a borrowed bank of the
    group's own (already act-read) PSUM tile; the DVE then adds the
    partial into an SBUF f32 accumulator.  No persistent num bank, no
    final PSUM->SBUF copy; the out DMA reads the SBUF accumulator.
  - Engine budget: PE 34.6us, ScalarE 30.7us, DVE 13us.
"""
import numpy as np
import ml_dtypes

import concourse.bacc as bacc
import concourse.tile as tile
from concourse import mybir, bass_utils

N_FULL = 4096
D = 512
N_CORES = 8
RP = N_FULL // N_CORES     # own rows per core = 512
P = 128
KC = D // P                # 4 contraction chunks of 128
JB = N_FULL // P           # 32 j-blocks
NPAIR = JB // 2            # 16 DR pairs for the num matmul
GRP = 4                    # j-blocks per ACTIVATE group
NGRP = JB // GRP           # 8 groups per matrix
CP = 112                   # classes 0..99, ones row at 100, zero pad
SHIFT = 4.9

F32 = mybir.dt.float32
BF16 = mybir.dt.bfloat16
F8 = mybir.dt.float8e4
AF = mybir.ActivationFunctionType
ALU = mybir.AluOpType
DR = mybir.MatmulPerfMode.DoubleRow

# xq DMA pieces (j-blocks per piece)
XQ_SPLIT = [3, 8, 8, 13]
# act-group sizes (j-blocks) per matrix: 3-jb groups keep the act
# (1536+312 cyc) inside the PE window (7.5 x 518 cyc); a small final group
# shortens the serial act->num->copy->DMA tail
GRPS0 = [3] * 10 + [2]
GRPS1 = [3] * 10 + [2]

_CACHE = {}


def _prefix(sizes):
    o = 0
    for n in sizes:
        yield o, n
        o += n


def _build(inv_t: float, bias_val: float):
    nc = bacc.Bacc("TRN2", target_bir_lowering=False, debug=False,
                   num_devices=N_CORES)

    # xq[p, jb, k, j'] = xn^T[k*128+p, jb*128+j']  (full xn)
    xq = nc.dram_tensor("xq", [P, JB * KC * P], F8, kind="ExternalInput")
    # wcc[p, m, k, i]: m=0 own pn^T cols, m=1 own xn^T cols
    wcc = nc.dram_tensor("wcc", [P, 2 * KC * RP], F8, kind="ExternalInput")
    # nmq[p, pair, kb, c] = nmx[c, (2*pair+kb)*128+p], nmx = [nm; ones; 0]
    nmq = nc.dram_tensor("nmq", [P, NPAIR * 2 * CP], F8, kind="ExternalInput")
    out = nc.dram_tensor("out", [CP, 2 * RP], F32, kind="ExternalOutput")

    with tile.TileContext(nc) as tc:
        with (
            tc.tile_pool(name="xpool", bufs=1) as xpool,
            tc.tile_pool(name="wpool", bufs=1) as wpool,
            tc.tile_pool(name="zpool", bufs=1) as zpool,
            tc.tile_pool(name="small", bufs=1) as small,
            tc.tile_pool(name="ps", bufs=2, space="PSUM") as pspool,
            tc.tile_pool(name="nps", bufs=2, space="PSUM") as npspool,
        ):
            # ---- constants ----
            dummy = small.tile([P, 1], F32, name="dummy")
            nc.vector.memset(dummy[:], 0.0)
            nc.scalar.activation(dummy[:], dummy[:], AF.Exp)
            bias_t = small.tile([P, 1], F32, name="bias_t")
            nc.vector.memset(bias_t[:], bias_val)
            zeros_w = small.tile([P, P], BF16, name="zeros_w")
            nc.vector.memset(zeros_w[:], 0.0)
            zeros_r = small.tile([P, 512], BF16, name="zeros_r")
            nc.vector.memset(zeros_r[:], 0.0)

            # ---- input tiles + DMAs, serial Sync kicks in order of first
            # use (per-DMA landing has a ~3.5us fixed floor, so fewer,
            # bigger pieces beat fine-grained splitting)
            xqt = [xpool.tile([P, n, KC, P], F8, name=f"xq{i}")
                   for i, n in enumerate(XQ_SPLIT)]
            pnt = wpool.tile([P, KC, RP], F8, name="pnt")
            xnt = wpool.tile([P, KC, RP], F8, name="xnt")
            nmqa = wpool.tile([P, 4, 2, CP], F8, name="nmqa")
            nmqb = wpool.tile([P, NPAIR - 4, 2, CP], F8, name="nmqb")
            zt = zpool.tile([P, 2 * JB, RP], F8, name="zt")

            offs = []
            o = 0
            for n in XQ_SPLIT:
                offs.append(o)
                o += n
            xq3 = xq.ap().rearrange("p (jb k j) -> p jb (k j)", jb=JB, k=KC)
            wc3 = wcc.ap().rearrange("p (m r) -> p m r", m=2)
            nm3 = nmq.ap().rearrange("p (u kb c) -> p u (kb c)",
                                     u=NPAIR, kb=2)

            def kick_xq(i):
                nc.sync.dma_start(
                    xqt[i][:].rearrange("p a b c -> p a (b c)"),
                    xq3[:, offs[i]:offs[i] + XQ_SPLIT[i]])

            nc.sync.dma_start(
                pnt[:].rearrange("p a b -> p (a b)"), wc3[:, 0])
            kick_xq(0)
            nc.sync.dma_start(
                nmqa[:].rearrange("p a b c -> p a (b c)"), nm3[:, 0:4])
            kick_xq(1)
            nc.sync.dma_start(
                nmqb[:].rearrange("p a b c -> p a (b c)"), nm3[:, 4:NPAIR])
            kick_xq(2)
            kick_xq(3)
            nc.sync.dma_start(
                xnt[:].rearrange("p a b -> p (a b)"), wc3[:, 1])

            def xq_slice(jb, b):
                for i, n in enumerate(XQ_SPLIT):
                    if jb < offs[i] + n:
                        return xqt[i][:, jb - offs[i], 2 * b:2 * b + 2, :]
                raise AssertionError

            def nm_slice(pair):
                if pair < 4:
                    return nmqa[:, pair, :, :]
                return nmqb[:, pair - 4, :, :]

            # units: (m, jb0, njb) act groups in stream order
            units = ([(0, j, n) for j, n in _prefix(GRPS0)]
                     + [(1, j, n) for j, n in _prefix(GRPS1)])
            num_ps = [npspool.tile([P, 512], F32, name=f"num{m}", tag="num")
                      for m in range(2)]
            ncopy = small.tile([P, 2, RP], F32, name="ncopy")

            def emit_num(idx):
                """num matmuls for all j-pairs newly completed by unit
                idx's act, into the matrix's persistent PSUM bank; copy
                out + DMA when the matrix completes."""
                m, jb0, njb = units[idx]
                pr0 = jb0 // 2            # pairs done before this unit
                prn = (jb0 + njb) // 2    # pairs done after it
                for pair in range(pr0, prn):
                    nc.tensor.matmul(
                        num_ps[m][:CP, :], nm_slice(pair),
                        zt[:, m * JB + 2 * pair:m * JB + 2 * pair + 2, :],
                        start=(pair == 0), stop=(pair == NPAIR - 1),
                        perf_mode=DR)
                if idx + 1 == len(units) or units[idx + 1][0] != m:
                    nc.vector.tensor_copy(ncopy[:CP, m, :], num_ps[m][:CP, :])
                    nc.sync.dma_start(out.ap()[:, m * RP:(m + 1) * RP],
                                      ncopy[:CP, m, :])

            pending = []
            for idx, (m, jb0, njb) in enumerate(units):
                ps = pspool.tile([P, 1536], F32, name=f"ps_{idx}", tag="ps")
                if idx == 0:
                    # HAM warm-up while the first DMAs land; must run
                    # contiguously into the real stream
                    for w in range(9):
                        nc.tensor.matmul(
                            ps[:, 0:512], zeros_w[:], zeros_r[:],
                            start=(w == 0), stop=(w == 8))
                for jl in range(njb):
                    jb = jb0 + jl
                    for b in range(2):
                        wt = pnt if m == 0 else xnt
                        rhs = wt[:, 2 * b:2 * b + 2, :]
                        nc.tensor.matmul(
                            ps[:, jl * 512:(jl + 1) * 512],
                            xq_slice(jb, b), rhs,
                            start=(b == 0), stop=(b == 1),
                            perf_mode=DR)
                u0 = m * JB + jb0
                nc.scalar.activation(
                    zt[:, u0:u0 + njb, :].rearrange("p a b -> p (a b)"),
                    ps[:, 0:njb * 512], AF.Exp, bias=bias_t[:], scale=inv_t)
                # num work trails its unit by TWO act-groups: the num
                # matmuls only read zt and write the persistent num banks
                # (no PSUM-rotation hazard), so a deeper delay keeps the
                # in-order PE queue entirely off the act's latency shadow
                pending.append(idx)
                if len(pending) > 2:
                    emit_num(pending.pop(0))
            for idx in pending:
                emit_num(idx)

    nc.compile()
    return nc


def _norm8(a):
    """fp8-quantize a row-normalized matrix, renormalizing once after
    quantization so self-similarity stays ~1 (keeps exp in fp8 range)."""
    eps = 1e-8
    an = a / np.maximum(np.linalg.norm(a, axis=-1, keepdims=True), eps)
    a8 = an.astype(ml_dtypes.float8_e4m3).astype(np.float32)
    a8 /= np.maximum(np.linalg.norm(a8, axis=-1, keepdims=True), eps)
    return a8.astype(ml_dtypes.float8_e4m3)


def make_in_maps(x, p, nmf, lab):
    xnT = np.ascontiguousarray(_norm8(x).T)   # [512, 4096] fp8
    pnT = np.ascontiguousarray(_norm8(p).T)
    # xq[p, jb, k, j'] = xnT[k*128+p, jb*128+j']
    xq = np.ascontiguousarray(
        xnT.reshape(KC, P, JB, P).transpose(1, 2, 0, 3)).reshape(P, -1)
    nmx = np.zeros((CP, N_FULL), np.float32)
    nmx[:100] = nmf
    nmx[100] = 1.0
    nmq = np.ascontiguousarray(
        nmx.T.reshape(NPAIR, 2, P, CP).transpose(2, 0, 1, 3)
    ).astype(ml_dtypes.float8_e4m3).reshape(P, -1)

    in_maps = []
    for c in range(N_CORES):
        rows = slice(c * RP, (c + 1) * RP)
        # wcc[p, m, k, i] = (pn|xn)T[k*128+p, rows]
        pncol = np.ascontiguousarray(
            pnT[:, rows].reshape(KC, P, RP).transpose(1, 0, 2)).reshape(P, -1)
        xncol = np.ascontiguousarray(
            xnT[:, rows].reshape(KC, P, RP).transpose(1, 0, 2)).reshape(P, -1)
        in_maps.append({
            "xq": xq,
            "wcc": np.concatenate([pncol, xncol], axis=1),
            "nmq": nmq,
        })
    return in_maps


def kernel(inst_embed, proxy, negative_mask, labels, temperature, margin):
    t = float(np.asarray(temperature))
    m = float(np.asarray(margin))
    inv_t = 1.0 / t
    bias_val = -m / t - SHIFT

    key = (t, m)
    if key not in _CACHE:
        _CACHE[key] = _build(inv_t, bias_val)
    nc = _CACHE[key]

    x = np.asarray(inst_embed, dtype=np.float32)
    p = np.asarray(proxy, dtype=np.float32)
    nmf = np.asarray(negative_mask, dtype=np.float32)
    lab = np.asarray(labels).astype(np.int64)

    in_maps = make_in_maps(x, p, nmf, lab)
    res = bass_utils.run_bass_kernel_spmd(nc, in_maps,
                                          core_ids=list(range(N_CORES)))
    idx = np.arange(RP)
    tot = 0.0
    for c in range(N_CORES):
        r = np.asarray(res.results[c]["out"]).astype(np.float64)
        lab_l = lab[c * RP:(c + 1) * RP]
        for m_i in range(2):
            blk = r[:, m_i * RP:(m_i + 1) * RP]
            den = blk[100, :]
            num = blk[lab_l, idx]
            tot += (np.log(den) - np.log(num)).sum()
    loss = -2.0 * np.log(t) + tot / N_FULL
    return np.float32(loss)


# revision 8
# speedup vs baseline: 1.2553x; 1.0075x over previous
"""Conditional contrastive loss on 8 TRN2 NeuronCores — transposed design.

Per core (512 own rows of inst_embed "x" / proxy "p", both matrices):
  - simT[j, i] with j on PSUM partitions: per j-block, lhsT = xn^T[jb]
    (stationary), rhs = own pn^T / xn^T columns (moving, 512 wide), fp8
    DoubleRow, K=512 = 2 DR instructions.
  - exp((simT - margin)/T - SHIFT) on ScalarE straight out of PSUM into
    fp8 SBUF z tiles, 4 j-blocks ([128, 2048]) per ACTIVATE.  fp8e4 here
    is IEEE-ish with max finite 240; quantized-renormalized unit vectors
    keep |sim| <= ~1.01 so z_max = e^(10.1-4.9) = 181 < 240.  The
    e^{-SHIFT} factor cancels in log(den) - log(num).
  - num/den on the tensor engine: mask[i,j] = nm[labels[i], j] is
    rank-100, so numC = [nm; ones] @ Z^T is an fp8 DR matmul stream
    contracting over j.  Row 100 (ones row) gives den; the host gathers
    num[i] = numC[labels[i], i].
  - PSUM is exactly 2 x [128, 2048] act groups (8 banks).  The num
    matmuls for each 4-jb group accumulate into a borrowed bank of the
    group's own (already act-read) PSUM tile; the DVE then adds the
    partial into an SBUF f32 accumulator.  No persistent num bank, no
    final PSUM->SBUF copy; the out DMA reads the SBUF accumulator.
  - Engine budget: PE 34.6us, ScalarE 30.7us, DVE 13us.
"""
import numpy as np
import ml_dtypes

import concourse.bacc as bacc
import concourse.tile as tile
from concourse import mybir, bass_utils

N_FULL = 4096
D = 512
N_CORES = 8
RP = N_FULL // N_CORES     # own rows per core = 512
P = 128
KC = D // P                # 4 contraction chunks of 128
JB = N_FULL // P           # 32 j-blocks
NPAIR = JB // 2            # 16 DR pairs for the num matmul
GRP = 4                    # j-blocks per ACTIVATE group
NGRP = JB // GRP           # 8 groups per matrix
CP = 112                   # classes 0..99, ones row at 100, zero pad
SHIFT = 4.9

F32 = mybir.dt.float32
BF16 = mybir.dt.bfloat16
F8 = mybir.dt.float8e4
AF = mybir.ActivationFunctionType
ALU = mybir.AluOpType
DR = mybir.MatmulPerfMode.DoubleRow

# xq DMA pieces (j-blocks per piece)
XQ_SPLIT = [3, 8, 8, 13]
# act-group sizes (j-blocks) per matrix: 3-jb groups keep the act
# (1536+312 cyc) inside the PE window (7.5 x 518 cyc); a small final group
# shortens the serial act->num->copy->DMA tail
GRPS0 = [3] * 10 + [2]
GRPS1 = [3] * 10 + [2]

_CACHE = {}


def _prefix(sizes):
    o = 0
    for n in sizes:
        yield o, n
        o += n


def _build(inv_t: float, bias_val: float):
    nc = bacc.Bacc("TRN2", target_bir_lowering=False, debug=False,
                   num_devices=N_CORES)

    # xq[p, jb, k, j'] = xn^T[k*128+p, jb*128+j']  (full xn)
    xq = nc.dram_tensor("xq", [P, JB * KC * P], F8, kind="ExternalInput")
    # wcc[p, m, k, i]: m=0 own pn^T cols, m=1 own xn^T cols
    wcc = nc.dram_tensor("wcc", [P, 2 * KC * RP], F8, kind="ExternalInput")
    # nmq[p, pair, kb, c] = nmx[c, (2*pair+kb)*128+p], nmx = [nm; ones; 0]
    nmq = nc.dram_tensor("nmq", [P, NPAIR * 2 * CP], F8, kind="ExternalInput")
    out = nc.dram_tensor("out", [CP, 2 * RP], F32, kind="ExternalOutput")

    with tile.TileContext(nc) as tc:
        with (
            tc.tile_pool(name="xpool", bufs=1) as xpool,
            tc.tile_pool(name="wpool", bufs=1) as wpool,
            tc.tile_pool(name="zpool", bufs=1) as zpool,
            tc.tile_pool(name="small", bufs=1) as small,
            tc.tile_pool(name="ps", bufs=2, space="PSUM") as pspool,
            tc.tile_pool(name="nps", bufs=2, space="PSUM") as npspool,
        ):
            # ---- constants ----
            dummy = small.tile([P, 1], F32, name="dummy")
            nc.vector.memset(dummy[:], 0.0)
            nc.scalar.activation(dummy[:], dummy[:], AF.Exp)
            bias_t = small.tile([P, 1], F32, name="bias_t")
            nc.vector.memset(bias_t[:], bias_val)
            zeros_w = small.tile([P, P], BF16, name="zeros_w")
            nc.vector.memset(zeros_w[:], 0.0)
            zeros_r = small.tile([P, 512], BF16, name="zeros_r")
            nc.vector.memset(zeros_r[:], 0.0)

            # ---- input tiles + DMAs, serial Sync kicks in order of first
            # use (per-DMA landing has a ~3.5us fixed floor, so fewer,
            # bigger pieces beat fine-grained splitting)
            xqt = [xpool.tile([P, n, KC, P], F8, name=f"xq{i}")
                   for i, n in enumerate(XQ_SPLIT)]
            pnt = wpool.tile([P, KC, RP], F8, name="pnt")
            xnt = wpool.tile([P, KC, RP], F8, name="xnt")
            nmqa = wpool.tile([P, 4, 2, CP], F8, name="nmqa")
            nmqb = wpool.tile([P, NPAIR - 4, 2, CP], F8, name="nmqb")
            zt = zpool.tile([P, 2 * JB, RP], F8, name="zt")

            offs = []
            o = 0
            for n in XQ_SPLIT:
                offs.append(o)
                o += n
            xq3 = xq.ap().rearrange("p (jb k j) -> p jb (k j)", jb=JB, k=KC)
            wc3 = wcc.ap().rearrange("p (m r) -> p m r", m=2)
            nm3 = nmq.ap().rearrange("p (u kb c) -> p u (kb c)",
                                     u=NPAIR, kb=2)

            def kick_xq(i):
                nc.sync.dma_start(
                    xqt[i][:].rearrange("p a b c -> p a (b c)"),
                    xq3[:, offs[i]:offs[i] + XQ_SPLIT[i]])

            nc.sync.dma_start(
                pnt[:].rearrange("p a b -> p (a b)"), wc3[:, 0])
            kick_xq(0)
            nc.sync.dma_start(
                nmqa[:].rearrange("p a b c -> p a (b c)"), nm3[:, 0:4])
            kick_xq(1)
            nc.sync.dma_start(
                nmqb[:].rearrange("p a b c -> p a (b c)"), nm3[:, 4:NPAIR])
            kick_xq(2)
            kick_xq(3)
            nc.sync.dma_start(
                xnt[:].rearrange("p a b -> p (a b)"), wc3[:, 1])

            def xq_slice(jb, b):
                for i, n in enumerate(XQ_SPLIT):
                    if jb < offs[i] + n:
                        return xqt[i][:, jb - offs[i], 2 * b:2 * b + 2, :]
                raise AssertionError

            def nm_slice(pair):
                if pair < 4:
                    return nmqa[:, pair, :, :]
                return nmqb[:, pair - 4, :, :]

            # units: (m, jb0, njb) act groups in stream order
            units = ([(0, j, n) for j, n in _prefix(GRPS0)]
                     + [(1, j, n) for j, n in _prefix(GRPS1)])
            num_ps = [npspool.tile([P, 512], F32, name=f"num{m}", tag="num")
                      for m in range(2)]
            ncopy = small.tile([P, 2, RP], F32, name="ncopy")

            def emit_num(idx):
                """num matmuls for all j-pairs newly completed by unit
                idx's act, into the matrix's persistent PSUM bank; copy
                out + DMA when the matrix completes."""
                m, jb0, njb = units[idx]
                pr0 = jb0 // 2            # pairs done before this unit
                prn = (jb0 + njb) // 2    # pairs done after it
                for pair in range(pr0, prn):
                    nc.tensor.matmul(
                        num_ps[m][:CP, :], nm_slice(pair),
                        zt[:, m * JB + 2 * pair:m * JB + 2 * pair + 2, :],
                        start=(pair == 0), stop=(pair == NPAIR - 1),
                        perf_mode=DR)
                if idx + 1 == len(units) or units[idx + 1][0] != m:
                    nc.vector.tensor_copy(ncopy[:CP, m, :], num_ps[m][:CP, :])
                    nc.sync.dma_start(out.ap()[:, m * RP:(m + 1) * RP],
                                      ncopy[:CP, m, :])

            pending = []
            for idx, (m, jb0, njb) in enumerate(units):
                ps = pspool.tile([P, 1536], F32, name=f"ps_{idx}", tag="ps")
                if idx == 0:
                    # HAM warm-up while the first DMAs land; must run
                    # contiguously into the real stream
                    for w in range(9):
                        nc.tensor.matmul(
                            ps[:, 0:512], zeros_w[:], zeros_r[:],
                            start=(w == 0), stop=(w == 8))
                for jl in range(njb):
                    jb = jb0 + jl
                    for b in range(2):
                        wt = pnt if m == 0 else xnt
                        rhs = wt[:, 2 * b:2 * b + 2, :]
                        nc.tensor.matmul(
                            ps[:, jl * 512:(jl + 1) * 512],
                            xq_slice(jb, b), rhs,
                            start=(b == 0), stop=(b == 1),
                            perf_mode=DR)
                u0 = m * JB + jb0
                nc.scalar.activation(
                    zt[:, u0:u0 + njb, :].rearrange("p a b -> p (a b)"),
                    ps[:, 0:njb * 512], AF.Exp, bias=bias_t[:], scale=inv_t)
                # num work trails its unit by TWO act-groups: the num
                # matmuls only read zt and write the persistent num banks
                # (no PSUM-rotation hazard), so a deeper delay keeps the
                # in-order PE queue entirely off the act's latency shadow
                pending.append(idx)
                if len(pending) > 3:
                    emit_num(pending.pop(0))
            for idx in pending:
                emit_num(idx)

    nc.compile()
    return nc


def _norm8(a):
    """fp8-quantize a row-normalized matrix, renormalizing once after
    quantization so self-similarity stays ~1 (keeps exp in fp8 range)."""
    eps = 1e-8
    an = a / np.maximum(np.linalg.norm(a, axis=-1, keepdims=True), eps)
    a8 = an.astype(ml_dtypes.float8_e4m3).astype(np.float32)
    a8 /= np.maximum(np.linalg.norm(a8, axis=-1, keepdims=True), eps)
    return a8.astype(ml_dtypes.float8_e4m3)


def make_in_maps(x, p, nmf, lab):
    xnT = np.ascontiguousarray(_norm8(x).T)   # [512, 4096] fp8
    pnT = np.ascontiguousarray(_norm8(p).T)
    # xq[p, jb, k, j'] = xnT[k*128+p, jb*128+j']
    xq = np.ascontiguousarray(
        xnT.reshape(KC, P, JB, P).transpose(1, 2, 0, 3)).reshape(P, -1)
    nmx = np.zeros((CP, N_FULL), np.float32)
    nmx[:100] = nmf
    nmx[100] = 1.0
    nmq = np.ascontiguousarray(
        nmx.T.reshape(NPAIR, 2, P, CP).transpose(2, 0, 1, 3)
    ).astype(ml_dtypes.float8_e4m3).reshape(P, -1)

    in_maps = []
    for c in range(N_CORES):
        rows = slice(c * RP, (c + 1) * RP)
        # wcc[p, m, k, i] = (pn|xn)T[k*128+p, rows]
        pncol = np.ascontiguousarray(
            pnT[:, rows].reshape(KC, P, RP).transpose(1, 0, 2)).reshape(P, -1)
        xncol = np.ascontiguousarray(
            xnT[:, rows].reshape(KC, P, RP).transpose(1, 0, 2)).reshape(P, -1)
        in_maps.append({
            "xq": xq,
            "wcc": np.concatenate([pncol, xncol], axis=1),
            "nmq": nmq,
        })
    return in_maps


def kernel(inst_embed, proxy, negative_mask, labels, temperature, margin):
    t = float(np.asarray(temperature))
    m = float(np.asarray(margin))
    inv_t = 1.0 / t
    bias_val = -m / t - SHIFT

    key = (t, m)
    if key not in _CACHE:
        _CACHE[key] = _build(inv_t, bias_val)
    nc = _CACHE[key]

    x = np.asarray(inst_embed, dtype=np.float32)
    p = np.asarray(proxy, dtype=np.float32)
    nmf = np.asarray(negative_mask, dtype=np.float32)
    lab = np.asarray(labels).astype(np.int64)

    in_maps = make_in_maps(x, p, nmf, lab)
    res = bass_utils.run_bass_kernel_spmd(nc, in_maps,
                                          core_ids=list(range(N_CORES)))
    idx = np.arange(RP)
    tot = 0.0
    for c in range(N_CORES):
        r = np.asarray(res.results[c]["out"]).astype(np.float64)
        lab_l = lab[c * RP:(c + 1) * RP]
        for m_i in range(2):
            blk = r[:, m_i * RP:(m_i + 1) * RP]
            den = blk[100, :]
            num = blk[lab_l, idx]
            tot += (np.log(den) - np.log(num)).sum()
    loss = -2.0 * np.log(t) + tot / N_FULL
    return np.float32(loss)


# revision 9
# speedup vs baseline: 1.2646x; 1.0074x over previous
"""Conditional contrastive loss on 8 TRN2 NeuronCores — transposed design.

Per core (512 own rows of inst_embed "x" / proxy "p", both matrices):
  - simT[j, i] with j on PSUM partitions: per j-block, lhsT = xn^T[jb]
    (stationary), rhs = own pn^T / xn^T columns (moving, 512 wide), fp8
    DoubleRow, K=512 = 2 DR instructions.
  - exp((simT - margin)/T - SHIFT) on ScalarE straight out of PSUM into
    fp8 SBUF z tiles, 4 j-blocks ([128, 2048]) per ACTIVATE.  fp8e4 here
    is IEEE-ish with max finite 240; quantized-renormalized unit vectors
    keep |sim| <= ~1.01 so z_max = e^(10.1-4.9) = 181 < 240.  The
    e^{-SHIFT} factor cancels in log(den) - log(num).
  - num/den on the tensor engine: mask[i,j] = nm[labels[i], j] is
    rank-100, so numC = [nm; ones] @ Z^T is an fp8 DR matmul stream
    contracting over j.  Row 100 (ones row) gives den; the host gathers
    num[i] = numC[labels[i], i].
  - PSUM is exactly 2 x [128, 2048] act groups (8 banks).  The num
    matmuls for each 4-jb group accumulate into a borrowed bank of the
    group's own (already act-read) PSUM tile; the DVE then adds the
    partial into an SBUF f32 accumulator.  No persistent num bank, no
    final PSUM->SBUF copy; the out DMA reads the SBUF accumulator.
  - Engine budget: PE 34.6us, ScalarE 30.7us, DVE 13us.
"""
import numpy as np
import ml_dtypes

import concourse.bacc as bacc
import concourse.tile as tile
from concourse import mybir, bass_utils

N_FULL = 4096
D = 512
N_CORES = 8
RP = N_FULL // N_CORES     # own rows per core = 512
P = 128
KC = D // P                # 4 contraction chunks of 128
JB = N_FULL // P           # 32 j-blocks
NPAIR = JB // 2            # 16 DR pairs for the num matmul
GRP = 4                    # j-blocks per ACTIVATE group
NGRP = JB // GRP           # 8 groups per matrix
CP = 112                   # classes 0..99, ones row at 100, zero pad
SHIFT = 4.9

F32 = mybir.dt.float32
BF16 = mybir.dt.bfloat16
F8 = mybir.dt.float8e4
AF = mybir.ActivationFunctionType
ALU = mybir.AluOpType
DR = mybir.MatmulPerfMode.DoubleRow

# xq DMA pieces (j-blocks per piece)
XQ_SPLIT = [3, 8, 8, 13]
# act-group sizes (j-blocks) per matrix: 3-jb groups keep the act
# (1536+312 cyc) inside the PE window (7.5 x 518 cyc); a small final group
# shortens the serial act->num->copy->DMA tail
GRPS0 = [3] * 10 + [2]
GRPS1 = [3] * 10 + [2]

_CACHE = {}


def _prefix(sizes):
    o = 0
    for n in sizes:
        yield o, n
        o += n


def _build(inv_t: float, bias_val: float):
    nc = bacc.Bacc("TRN2", target_bir_lowering=False, debug=False,
                   num_devices=N_CORES)

    # xq[p, jb, k, j'] = xn^T[k*128+p, jb*128+j']  (full xn)
    xq = nc.dram_tensor("xq", [P, JB * KC * P], F8, kind="ExternalInput")
    # wcc[p, m, k, i]: m=0 own pn^T cols, m=1 own xn^T cols
    wcc = nc.dram_tensor("wcc", [P, 2 * KC * RP], F8, kind="ExternalInput")
    # nmq[p, pair, kb, c] = nmx[c, (2*pair+kb)*128+p], nmx = [nm; ones; 0]
    nmq = nc.dram_tensor("nmq", [P, NPAIR * 2 * CP], F8, kind="ExternalInput")
    out = nc.dram_tensor("out", [CP, 2 * RP], BF16, kind="ExternalOutput")

    with tile.TileContext(nc) as tc:
        with (
            tc.tile_pool(name="xpool", bufs=1) as xpool,
            tc.tile_pool(name="wpool", bufs=1) as wpool,
            tc.tile_pool(name="zpool", bufs=1) as zpool,
            tc.tile_pool(name="small", bufs=1) as small,
            tc.tile_pool(name="ps", bufs=2, space="PSUM") as pspool,
            tc.tile_pool(name="nps", bufs=2, space="PSUM") as npspool,
        ):
            # ---- constants ----
            dummy = small.tile([P, 1], F32, name="dummy")
            nc.vector.memset(dummy[:], 0.0)
            nc.scalar.activation(dummy[:], dummy[:], AF.Exp)
            bias_t = small.tile([P, 1], F32, name="bias_t")
            nc.vector.memset(bias_t[:], bias_val)
            zeros_w = small.tile([P, P], BF16, name="zeros_w")
            nc.vector.memset(zeros_w[:], 0.0)
            zeros_r = small.tile([P, 512], BF16, name="zeros_r")
            nc.vector.memset(zeros_r[:], 0.0)

            # ---- input tiles + DMAs, serial Sync kicks in order of first
            # use (per-DMA landing has a ~3.5us fixed floor, so fewer,
            # bigger pieces beat fine-grained splitting)
            xqt = [xpool.tile([P, n, KC, P], F8, name=f"xq{i}")
                   for i, n in enumerate(XQ_SPLIT)]
            pnt = wpool.tile([P, KC, RP], F8, name="pnt")
            xnt = wpool.tile([P, KC, RP], F8, name="xnt")
            nmqa = wpool.tile([P, 4, 2, CP], F8, name="nmqa")
            nmqb = wpool.tile([P, NPAIR - 4, 2, CP], F8, name="nmqb")
            zt = zpool.tile([P, 2 * JB, RP], F8, name="zt")

            offs = []
            o = 0
            for n in XQ_SPLIT:
                offs.append(o)
                o += n
            xq3 = xq.ap().rearrange("p (jb k j) -> p jb (k j)", jb=JB, k=KC)
            wc3 = wcc.ap().rearrange("p (m r) -> p m r", m=2)
            nm3 = nmq.ap().rearrange("p (u kb c) -> p u (kb c)",
                                     u=NPAIR, kb=2)

            def kick_xq(i):
                nc.sync.dma_start(
                    xqt[i][:].rearrange("p a b c -> p a (b c)"),
                    xq3[:, offs[i]:offs[i] + XQ_SPLIT[i]])

            nc.sync.dma_start(
                pnt[:].rearrange("p a b -> p (a b)"), wc3[:, 0])
            kick_xq(0)
            nc.sync.dma_start(
                nmqa[:].rearrange("p a b c -> p a (b c)"), nm3[:, 0:4])
            kick_xq(1)
            nc.sync.dma_start(
                nmqb[:].rearrange("p a b c -> p a (b c)"), nm3[:, 4:NPAIR])
            kick_xq(2)
            kick_xq(3)
            nc.sync.dma_start(
                xnt[:].rearrange("p a b -> p (a b)"), wc3[:, 1])

            def xq_slice(jb, b):
                for i, n in enumerate(XQ_SPLIT):
                    if jb < offs[i] + n:
                        return xqt[i][:, jb - offs[i], 2 * b:2 * b + 2, :]
                raise AssertionError

            def nm_slice(pair):
                if pair < 4:
                    return nmqa[:, pair, :, :]
                return nmqb[:, pair - 4, :, :]

            # units: (m, jb0, njb) act groups in stream order
            units = ([(0, j, n) for j, n in _prefix(GRPS0)]
                     + [(1, j, n) for j, n in _prefix(GRPS1)])
            num_ps = [npspool.tile([P, 512], F32, name=f"num{m}", tag="num")
                      for m in range(2)]
            ncopy = small.tile([P, 2, RP], BF16, name="ncopy")

            def emit_num(idx):
                """num matmuls for all j-pairs newly completed by unit
                idx's act, into the matrix's persistent PSUM bank; copy
                out + DMA when the matrix completes."""
                m, jb0, njb = units[idx]
                pr0 = jb0 // 2            # pairs done before this unit
                prn = (jb0 + njb) // 2    # pairs done after it
                for pair in range(pr0, prn):
                    nc.tensor.matmul(
                        num_ps[m][:CP, :], nm_slice(pair),
                        zt[:, m * JB + 2 * pair:m * JB + 2 * pair + 2, :],
                        start=(pair == 0), stop=(pair == NPAIR - 1),
                        perf_mode=DR)
                if idx + 1 == len(units) or units[idx + 1][0] != m:
                    nc.vector.tensor_copy(ncopy[:CP, m, :], num_ps[m][:CP, :])
                    nc.sync.dma_start(out.ap()[:, m * RP:(m + 1) * RP],
                                      ncopy[:CP, m, :])

            pending = []
            for idx, (m, jb0, njb) in enumerate(units):
                ps = pspool.tile([P, 1536], F32, name=f"ps_{idx}", tag="ps")
                if idx == 0:
                    # HAM warm-up while the first DMAs land; must run
                    # contiguously into the real stream
                    for w in range(9):
                        nc.tensor.matmul(
                            ps[:, 0:512], zeros_w[:], zeros_r[:],
                            start=(w == 0), stop=(w == 8))
                for jl in range(njb):
                    jb = jb0 + jl
                    for b in range(2):
                        wt = pnt if m == 0 else xnt
                        rhs = wt[:, 2 * b:2 * b + 2, :]
                        nc.tensor.matmul(
                            ps[:, jl * 512:(jl + 1) * 512],
                            xq_slice(jb, b), rhs,
                            start=(b == 0), stop=(b == 1),
                            perf_mode=DR)
                u0 = m * JB + jb0
                nc.scalar.activation(
                    zt[:, u0:u0 + njb, :].rearrange("p a b -> p (a b)"),
                    ps[:, 0:njb * 512], AF.Exp, bias=bias_t[:], scale=inv_t)
                # num work trails its unit by TWO act-groups: the num
                # matmuls only read zt and write the persistent num banks
                # (no PSUM-rotation hazard), so a deeper delay keeps the
                # in-order PE queue entirely off the act's latency shadow
                pending.append(idx)
                if len(pending) > 3:
                    emit_num(pending.pop(0))
            for idx in pending:
                emit_num(idx)

    nc.compile()
    return nc


def _norm8(a):
    """fp8-quantize a row-normalized matrix, renormalizing once after
    quantization so self-similarity stays ~1 (keeps exp in fp8 range)."""
    eps = 1e-8
    an = a / np.maximum(np.linalg.norm(a, axis=-1, keepdims=True), eps)
    a8 = an.astype(ml_dtypes.float8_e4m3).astype(np.float32)
    a8 /= np.maximum(np.linalg.norm(a8, axis=-1, keepdims=True), eps)
    return a8.astype(ml_dtypes.float8_e4m3)


def make_in_maps(x, p, nmf, lab):
    xnT = np.ascontiguousarray(_norm8(x).T)   # [512, 4096] fp8
    pnT = np.ascontiguousarray(_norm8(p).T)
    # xq[p, jb, k, j'] = xnT[k*128+p, jb*128+j']
    xq = np.ascontiguousarray(
        xnT.reshape(KC, P, JB, P).transpose(1, 2, 0, 3)).reshape(P, -1)
    nmx = np.zeros((CP, N_FULL), np.float32)
    nmx[:100] = nmf
    nmx[100] = 1.0
    nmq = np.ascontiguousarray(
        nmx.T.reshape(NPAIR, 2, P, CP).transpose(2, 0, 1, 3)
    ).astype(ml_dtypes.float8_e4m3).reshape(P, -1)

    in_maps = []
    for c in range(N_CORES):
        rows = slice(c * RP, (c + 1) * RP)
        # wcc[p, m, k, i] = (pn|xn)T[k*128+p, rows]
        pncol = np.ascontiguousarray(
            pnT[:, rows].reshape(KC, P, RP).transpose(1, 0, 2)).reshape(P, -1)
        xncol = np.ascontiguousarray(
            xnT[:, rows].reshape(KC, P, RP).transpose(1, 0, 2)).reshape(P, -1)
        in_maps.append({
            "xq": xq,
            "wcc": np.concatenate([pncol, xncol], axis=1),
            "nmq": nmq,
        })
    return in_maps


def kernel(inst_embed, proxy, negative_mask, labels, temperature, margin):
    t = float(np.asarray(temperature))
    m = float(np.asarray(margin))
    inv_t = 1.0 / t
    bias_val = -m / t - SHIFT

    key = (t, m)
    if key not in _CACHE:
        _CACHE[key] = _build(inv_t, bias_val)
    nc = _CACHE[key]

    x = np.asarray(inst_embed, dtype=np.float32)
    p = np.asarray(proxy, dtype=np.float32)
    nmf = np.asarray(negative_mask, dtype=np.float32)
    lab = np.asarray(labels).astype(np.int64)

    in_maps = make_in_maps(x, p, nmf, lab)
    res = bass_utils.run_bass_kernel_spmd(nc, in_maps,
                                          core_ids=list(range(N_CORES)))
    idx = np.arange(RP)
    tot = 0.0
    for c in range(N_CORES):
        r = np.asarray(res.results[c]["out"]).astype(np.float64)
        lab_l = lab[c * RP:(c + 1) * RP]
        for m_i in range(2):
            blk = r[:, m_i * RP:(m_i + 1) * RP]
            den = blk[100, :]
            num = blk[lab_l, idx]
            tot += (np.log(den) - np.log(num)).sum()
    loss = -2.0 * np.log(t) + tot / N_FULL
    return np.float32(loss)
